# revision 19
# baseline (speedup 1.0000x reference)
"""GCN forward (gather + segment-sum + matmul) on 8 TRN2 NeuronCores.

Algorithm (factorized GCN):
    out[i] = deg[i] * (sum_{j in N(i)} deg[j] * X[j]) @ W

Sharding: destination nodes are split across the 8 cores (12500 rows each);
the fp16 feature table Y = deg[:,None]*X (GCN norm folded in, like the
reference's precomputed rsqrt degrees) is replicated to every core's HBM.
Each core:
  - gathers the fp16 rows of Y for its ~200K edges with gpsimd dma_gather
    (the memory-bound part; int16 indices force a 4-way chunking of the
    100K-row table, so each core keeps 4 chunk-local edge streams),
  - window membership is STEERED on the host (_steer_windows) so per-(chunk,
    window) edge counts pack tightly into 4x128 tiles across all 8 SPMD
    cores (~11% padding vs ~25% for contiguous windows),
  - builds one-hot selection matrices sel[e,d] = (dstrel[e] == d) in batches
    of 16 tiles with one broadcast-AP DVE is_equal,
  - segment-sums via TensorE: A_T[f,d] += G[e,f]^T @ sel[e,d], accumulating
    in PSUM over all of a 128-destination window's edge tiles (across the 4
    chunk streams),
  - applies W with a second matmul and scales rows by deg_dest,
  - writes its 12544-row slice; the host unpermutes the steered windows.

The aggregation, both matmuls and the dest-degree scaling happen on device;
the host computes indices/partitioning/normalization staging.
"""
import os

import numpy as np

N = 100000
E = 1600000
F = 128
P = 128
NCORES = 8
NPC = N // NCORES          # 12500 destination rows per core
NW = (NPC + P - 1) // P    # 98 windows of 128 destinations per core
NQ = 4                     # table chunks (int16 gather indices)
CHUNK = 25000              # rows per chunk
GB_TILES = int(os.environ.get("GCN_GB_TILES", "16"))  # tiles per gather call
# dma_gather per-call num_idxs is capped by the SWDGE descriptor-ring carveout,
# sized via Bacc(dynamic_dma_scratch_size=...): 16*128=2048 idxs (128 descs/lane)
# is safe with the 64KB carveout below; bigger calls amortize the ~1us/call Q7
# launch and the ~2-3us inter-drain bubble on each of the 4 SWDGE queues.

_PROGRAM_CACHE: dict = {}


def _row_ids_from_pointers(row_pointers: np.ndarray) -> np.ndarray:
    """Replicates jnp.repeat(arange(N), diff(rp), total_repeat_length=E)."""
    rl = np.diff(row_pointers.astype(np.int64))
    starts = np.concatenate([np.zeros(1, np.int64), np.cumsum(rl)[:-1]])
    return np.searchsorted(starts, np.arange(E, dtype=np.int64), side="right") - 1


NBIG = int(os.environ.get("GCN_NBIG", "24"))  # overflow windows for steering


def _steer_windows(dmat, nbig=NBIG, nw=NW, wsize=P):
    """Assign destination rows to windows so per-(chunk, window) edge counts
    pack tightly under 4*128, minimizing SPMD tile padding. Small windows are
    hard-capped at 4 tiles; overflow rows go to the trailing big windows."""
    npc = dmat.shape[0]
    slots = np.full(nw, wsize, np.int64)
    slots[-1] -= nw * wsize - npc
    nsmall = nw - nbig
    capv = 4 * wsize
    order = np.argsort(-dmat.sum(1), kind="stable")
    S = np.zeros((nw, NQ), np.int64)
    used = np.zeros(nw, np.int64)
    w_of = np.empty(npc, np.int64)
    slot_of = np.empty(npc, np.int64)
    for i in order:
        di = dmat[i]
        frees = used[:nsmall] < slots[:nsmall]
        fits = frees & np.all(S[:nsmall] + di <= capv, axis=1)
        cand = np.flatnonzero(fits)
        if cand.size:
            rem = (slots[cand] - used[cand]).astype(np.float64)
            perslot = (capv - S[cand] - di).min(1) / np.maximum(rem - 1, 0.5)
            w = cand[np.argmax(perslot)]
        else:
            bfree = np.flatnonzero(used[nsmall:] < slots[nsmall:]) + nsmall
            if bfree.size == 0:
                cand2 = np.flatnonzero(used < slots)
                add = (-(-(S[cand2] + di) // wsize) - (-(-S[cand2] // wsize))).sum(1)
                w = cand2[np.argmin(add)]
            else:
                w = bfree[np.argmin(S[bfree].max(1))]
        w_of[i] = w
        slot_of[i] = used[w]
        S[w] += di
        used[w] += 1
    return w_of, slot_of


def _preprocess(X, weight, degrees, row_pointers, column_index):
    row_ids = _row_ids_from_pointers(row_pointers)          # [E] sorted, in [0,N)
    col = column_index.astype(np.int64)
    deg = np.ascontiguousarray(degrees.astype(np.float32))

    core = row_ids // NPC                                   # [E] in [0,8)
    local = row_ids - core * NPC
    q = col // CHUNK                                        # [E] in [0,4)

    # per-(core, local row, chunk) edge counts for window steering
    dkey = (core * NPC + local) * NQ + q
    dmat = np.bincount(dkey, minlength=NCORES * NPC * NQ).reshape(NCORES, NPC, NQ)
    W_OF = np.empty((NCORES, NPC), np.int64)
    SLOT_OF = np.empty((NCORES, NPC), np.int64)
    for c in range(NCORES):
        W_OF[c], SLOT_OF[c] = _steer_windows(dmat[c])

    w_local = W_OF[core, local]                             # [E] in [0,98)
    dstrel_all = SLOT_OF[core, local].astype(np.float32)
    src16_all = (col - q * CHUNK).astype(np.int16)

    key = ((core * NQ + q) * NW + w_local).astype(np.int64)  # (c, q, w)
    counts = np.bincount(key, minlength=NCORES * NQ * NW).reshape(NCORES, NQ, NW)
    t_qw = -(-counts.max(axis=0) // P)                       # [NQ, NW]
    # no chunk may have an empty stream (zero-size params break AP lowering);
    # a pad tile (src=0, dstrel=-1) contributes nothing
    for qq in range(NQ):
        if t_qw[qq].sum() == 0:
            t_qw[qq, 0] = 1
    lq = t_qw.sum(axis=1) * P                                # [NQ] stream lengths
    chunk_base = np.concatenate([np.zeros(1, np.int64), np.cumsum(lq)])
    ltot = int(chunk_base[-1])
    # offset of window w's padded segment within chunk q's stream
    offs_qw = np.cumsum(np.concatenate([np.zeros((NQ, 1), np.int64), t_qw[:, :-1]], axis=1) * P, axis=1) \
        if False else (np.cumsum(t_qw, axis=1) - t_qw) * P   # [NQ, NW] exclusive prefix

    order = np.argsort(key, kind="stable")
    key_s = key[order]
    starts_flat = np.concatenate([np.zeros(1, np.int64), np.cumsum(counts.reshape(-1))])[:-1]
    rank_s = np.arange(E, dtype=np.int64) - starts_flat[key_s]
    q_s = (key_s // NW) % NQ
    w_s = key_s % NW
    core_s = key_s // (NQ * NW)
    pos_s = chunk_base[q_s] + offs_qw[q_s, w_s] + rank_s     # [E] position in core's array

    src_pad = np.zeros((NCORES, ltot), np.int16)
    dstrel_pad = np.full((NCORES, ltot), -1.0, np.float32)
    src_pad[core_s, pos_s] = src16_all[order]
    dstrel_pad[core_s, pos_s] = dstrel_all[order]

    # per-chunk device layouts
    idx_w, dst_t = [], []
    for qq in range(NQ):
        sl = slice(int(chunk_base[qq]), int(chunk_base[qq + 1]))
        s = src_pad[:, sl]                                   # [NC, LQ]
        # wrapped idx layout [128, LQ/16]: idx i at [i%16, i//16], replicated 8x
        iw = np.tile(s.reshape(NCORES, -1, 16).transpose(0, 2, 1), (1, 8, 1))
        idx_w.append(np.ascontiguousarray(iw))
        dst_t.append(np.ascontiguousarray(
            dstrel_pad[:, sl].reshape(NCORES, -1, P).transpose(0, 2, 1).astype(np.float16)))

    # per-core dest-degree table [P, NW] in steered window order
    degt = np.zeros((NCORES, P, NW), np.float32)
    for c in range(NCORES):
        degt[c, SLOT_OF[c], W_OF[c]] = deg[c * NPC : (c + 1) * NPC]
    # map original local row -> position in the core's steered output
    pos = W_OF * P + SLOT_OF                                # [NCORES, NPC]

    # fold the source-degree normalization into the gather table (standard
    # GCN norm precompute, same class as the reference's rsqrt degrees):
    # gathered rows arrive pre-scaled, so sel is a pure one-hot (1 DVE op).
    xt = np.ascontiguousarray((deg[:, None] * X).astype(np.float16))
    w16 = np.ascontiguousarray(weight.astype(np.float16))
    t_key = tuple(tuple(int(x) for x in row) for row in t_qw)
    return xt, w16, idx_w, dst_t, degt, pos, t_key


SB_T = int(os.environ.get("GCN_SB_T", "16"))  # tiles per batched sel build


def _build_program(t_qw):
    import concourse.bacc as bacc
    import concourse.bass as bass
    import concourse.mybir as mybir
    import concourse.tile as tile

    lq = [sum(t_qw[q]) * P for q in range(NQ)]

    nc = bacc.Bacc(
        "TRN2", target_bir_lowering=False, num_swdge_queues=4,
        # descriptor-ring carveout: 2x the default so a queue can hold two
        # gather calls' descriptors -> descgen of call n+1 overlaps drain of n
        dynamic_dma_scratch_size=int(os.environ.get("GCN_DDS", "65536")),
    )
    xt_p = nc.declare_dram_parameter("xt", [N, F], mybir.dt.float16, isOutput=False)
    idx_ps = [nc.declare_dram_parameter(f"idx{q}", [P, lq[q] // 16], mybir.dt.int16, isOutput=False) for q in range(NQ)]
    dst_ps = [nc.declare_dram_parameter(f"dstrel{q}", [P, lq[q] // P], mybir.dt.float16, isOutput=False) for q in range(NQ)]
    degt_p = nc.declare_dram_parameter("degt", [P, NW], mybir.dt.float32, isOutput=False)
    w_p = nc.declare_dram_parameter("w16", [F, F], mybir.dt.float16, isOutput=False)
    out_p = nc.declare_dram_parameter("out", [NW * P, F], mybir.dt.float32, isOutput=True)

    def bcast_mid(ap, t):
        # [128, t] AP -> [128, t, F] with stride-0 inner (value per (p, tile))
        return bass.AP(ap.tensor, ap.offset, [ap.ap[0], [ap.ap[1][0], t], [0, F]])

    with tile.TileContext(nc) as tc:
        with (
            tc.tile_pool(name="persist", bufs=1) as persist,
            tc.tile_pool(name="gblk", bufs=int(os.environ.get("GCN_GBUFS", "3"))) as gpool,
            tc.tile_pool(name="selp", bufs=int(os.environ.get("GCN_SBUFS", "2"))) as selpool,
            tc.tile_pool(name="atsb", bufs=2) as atpool,
            tc.tile_pool(name="outsb", bufs=2) as outpool,
            tc.tile_pool(name="psum1", bufs=2, space="PSUM") as psum1,
            tc.tile_pool(name="psum2", bufs=2, space="PSUM") as psum2,
        ):
            # Stage idx/dst/ds in call-aligned pieces so the first gather of
            # each queue only waits on a small initial load, not the full
            # ~1MB table (ramp was ~29us with whole-table loads).
            IDX_PIECE = GB_TILES * P // 16 * 8      # 8 gather-calls per piece
            SEL_PIECE = SB_T * 16                   # 16 sel-batches per piece
            idx_sb, dst_sb = [], []
            for q in range(NQ):
                idx_sb.append(persist.tile([P, lq[q] // 16], mybir.dt.int16,
                                           tag=f"idx{q}", name=f"idx{q}"))
                dst_sb.append(persist.tile([P, lq[q] // P], mybir.dt.float16,
                                           tag=f"dst{q}", name=f"dst{q}"))
            np_idx = max(-(-(lq[q] // 16) // IDX_PIECE) for q in range(NQ))
            np_sel = max(-(-(lq[q] // P) // SEL_PIECE) for q in range(NQ))
            for pi in range(max(np_idx, np_sel)):
                for q in range(NQ):
                    a, b = pi * IDX_PIECE, min((pi + 1) * IDX_PIECE, lq[q] // 16)
                    if a < b:
                        nc.sync.dma_start(idx_sb[q][:, a:b], idx_ps[q][:, a:b])
                    a, b = pi * SEL_PIECE, min((pi + 1) * SEL_PIECE, lq[q] // P)
                    if a < b:
                        nc.sync.dma_start(dst_sb[q][:, a:b], dst_ps[q][:, a:b])
            degt_sb = persist.tile([P, NW], mybir.dt.float32)
            nc.sync.dma_start(degt_sb[:], degt_p[:])
            w_sb = persist.tile([F, F], mybir.dt.float16)
            nc.sync.dma_start(w_sb[:], w_p[:])
            c_i32 = persist.tile([P, P], mybir.dt.int32)
            nc.gpsimd.iota(c_i32[:], pattern=[[1, P]], base=0, channel_multiplier=0)
            c_f16 = persist.tile([P, P], mybir.dt.float16)
            nc.vector.tensor_copy(c_f16[:], c_i32[:])
            zero_sb = persist.tile([P, F], mybir.dt.float32)
            nc.vector.memset(zero_sb[:], 0.0)

            pos = [0] * NQ
            gblk = [None] * NQ
            selblk = [None] * NQ
            for w in range(NW):
                ntiles_w = sum(t_qw[q][w] for q in range(NQ))
                if ntiles_w == 0:
                    nc.sync.dma_start(out=out_p[w * P : (w + 1) * P, :], in_=zero_sb[:])
                    continue
                at_ps = psum1.tile([F, P], mybir.dt.float32, space="PSUM")
                k = 0
                for q in range(NQ):
                    for _t in range(t_qw[q][w]):
                        if pos[q] % GB_TILES == 0:
                            nt_call = min(GB_TILES, lq[q] // P - pos[q])
                            nidx = nt_call * P
                            gblk[q] = gpool.tile(
                                [P, GB_TILES * F], mybir.dt.float16,
                                tag=f"gblk{q}", name=f"gblk{q}",
                            )
                            nc.gpsimd.dma_gather(
                                out_ap=gblk[q][:, : nt_call * F].rearrange(
                                    "p (k f) -> p k f", f=F
                                ),
                                in_ap=xt_p[q * CHUNK : (q + 1) * CHUNK, :],
                                idxs_ap=idx_sb[q][:, pos[q] * P // 16 : (pos[q] * P + nidx) // 16],
                                num_idxs=nidx,
                                num_idxs_reg=nidx,
                                elem_size=F,
                                queue_num=q,
                                single_packet=(os.environ.get('GCN_SP','0')=='1'),
                            )
                        if pos[q] % SB_T == 0:
                            nt_s = min(SB_T, lq[q] // P - pos[q])
                            selblk[q] = selpool.tile(
                                [P, SB_T * F], mybir.dt.float16,
                                tag=f"sel{q}", name=f"sel{q}",
                            )
                            c_b = bass.AP(c_f16[:].tensor, c_f16[:].offset,
                                          [c_f16[:].ap[0], [0, nt_s], [1, F]])
                            nc.vector.tensor_tensor(
                                out=selblk[q][:, : nt_s * F].rearrange("p (t f) -> p t f", f=F),
                                in0=c_b,
                                in1=bcast_mid(dst_sb[q][:, pos[q] : pos[q] + nt_s], nt_s),
                                op=mybir.AluOpType.is_equal,
                            )
                        j = pos[q] % GB_TILES
                        js = pos[q] % SB_T
                        nc.tensor.matmul(
                            out=at_ps[:],
                            lhsT=gblk[q][:, j * F : (j + 1) * F],
                            rhs=selblk[q][:, js * F : (js + 1) * F],
                            start=(k == 0),
                            stop=(k == ntiles_w - 1),
                        )
                        pos[q] += 1
                        k += 1
                at_sb = atpool.tile([F, P], mybir.dt.float16)
                nc.scalar.activation(at_sb[:], at_ps[:], mybir.ActivationFunctionType.Copy)
                o2_ps = psum2.tile([P, F], mybir.dt.float32, space="PSUM")
                nc.tensor.matmul(out=o2_ps[:], lhsT=at_sb[:], rhs=w_sb[:], start=True, stop=True)
                outsb = outpool.tile([P, F], mybir.dt.float32)
                nc.scalar.activation(outsb[:], o2_ps[:], mybir.ActivationFunctionType.Copy,
                                     scale=degt_sb[:, w : w + 1])
                nc.sync.dma_start(out=out_p[w * P : (w + 1) * P, :], in_=outsb[:])
    nc.compile()
    return nc


def _get_program(t_key):
    key = (t_key, GB_TILES, SB_T)
    if key not in _PROGRAM_CACHE:
        _PROGRAM_CACHE[key] = _build_program(t_key)
    return _PROGRAM_CACHE[key]


def _run(nc, in_maps, trace=False, **kw):
    from concourse.bass_utils import run_bass_kernel_spmd

    return run_bass_kernel_spmd(nc, in_maps, core_ids=list(range(NCORES)),
                                trace=trace, **kw)


def kernel(X, weight, degrees, row_pointers, column_index, _trace=False, _ret_raw=False):
    assert X.shape == (N, F) and column_index.shape == (E,)
    xt, w16, idx_w, dst_t, degt, pos, t_key = _preprocess(
        X, weight, degrees, row_pointers, column_index
    )
    nc = _get_program(t_key)
    in_maps = []
    for c in range(NCORES):
        m = {"xt": xt, "degt": degt[c], "w16": w16}
        for q in range(NQ):
            m[f"idx{q}"] = idx_w[q][c]
            m[f"dstrel{q}"] = dst_t[q][c]
        in_maps.append(m)
    res = _run(nc, in_maps, trace=_trace)
    out = np.empty((N, F), np.float32)
    for c in range(NCORES):
        out[c * NPC : (c + 1) * NPC] = res.results[c]["out"][pos[c]]
    if _ret_raw:
        return out, res
    return out



# revision 20
# speedup vs baseline: 1.0545x; 1.0545x over previous
"""GCN forward (gather + segment-sum + matmul) on 8 TRN2 NeuronCores.

Algorithm (factorized GCN):
    out[i] = deg[i] * (sum_{j in N(i)} deg[j] * X[j]) @ W

Sharding: destination nodes are split across the 8 cores (12500 rows each);
the fp16 feature table Y = deg[:,None]*X (GCN norm folded in, like the
reference's precomputed rsqrt degrees) is replicated to every core's HBM.
Each core:
  - gathers the fp16 rows of Y for its ~200K edges with gpsimd dma_gather
    (the memory-bound part; int16 indices force a 4-way chunking of the
    100K-row table, so each core keeps 4 chunk-local edge streams),
  - window membership is STEERED on the host (_steer_windows) so per-(chunk,
    window) edge counts pack tightly into 4x128 tiles across all 8 SPMD
    cores (~11% padding vs ~25% for contiguous windows),
  - builds one-hot selection matrices sel[e,d] = (dstrel[e] == d) in batches
    of 16 tiles with one broadcast-AP DVE is_equal,
  - segment-sums via TensorE: A_T[f,d] += G[e,f]^T @ sel[e,d], accumulating
    in PSUM over all of a 128-destination window's edge tiles (across the 4
    chunk streams),
  - applies W with a second matmul and scales rows by deg_dest,
  - writes its 12544-row slice; the host unpermutes the steered windows.

The aggregation, both matmuls and the dest-degree scaling happen on device;
the host computes indices/partitioning/normalization staging.
"""
import os

import numpy as np

N = 100000
E = 1600000
F = 128
P = 128
NCORES = 8
NPC = N // NCORES          # 12500 destination rows per core
NW = (NPC + P - 1) // P    # 98 windows of 128 destinations per core
NQ = 4                     # table chunks (int16 gather indices)
CHUNK = 25000              # rows per chunk
GB_TILES = int(os.environ.get("GCN_GB_TILES", "16"))  # tiles per gather call
# dma_gather per-call num_idxs is capped by the SWDGE descriptor-ring carveout,
# sized via Bacc(dynamic_dma_scratch_size=...): 16*128=2048 idxs (128 descs/lane)
# is safe with the 64KB carveout below; bigger calls amortize the ~1us/call Q7
# launch and the ~2-3us inter-drain bubble on each of the 4 SWDGE queues.

_PROGRAM_CACHE: dict = {}


def _row_ids_from_pointers(row_pointers: np.ndarray) -> np.ndarray:
    """Replicates jnp.repeat(arange(N), diff(rp), total_repeat_length=E)."""
    rl = np.diff(row_pointers.astype(np.int64))
    starts = np.concatenate([np.zeros(1, np.int64), np.cumsum(rl)[:-1]])
    return np.searchsorted(starts, np.arange(E, dtype=np.int64), side="right") - 1


NBIG = int(os.environ.get("GCN_NBIG", "24"))  # overflow windows for steering


def _steer_windows(dmat, nbig=NBIG, nw=NW, wsize=P):
    """Assign destination rows to windows so per-(chunk, window) edge counts
    pack tightly under 4*128, minimizing SPMD tile padding. Small windows are
    hard-capped at 4 tiles; overflow rows go to the trailing big windows."""
    npc = dmat.shape[0]
    slots = np.full(nw, wsize, np.int64)
    slots[-1] -= nw * wsize - npc
    nsmall = nw - nbig
    capv = 4 * wsize
    order = np.argsort(-dmat.sum(1), kind="stable")
    S = np.zeros((nw, NQ), np.int64)
    used = np.zeros(nw, np.int64)
    w_of = np.empty(npc, np.int64)
    slot_of = np.empty(npc, np.int64)
    for i in order:
        di = dmat[i]
        frees = used[:nsmall] < slots[:nsmall]
        fits = frees & np.all(S[:nsmall] + di <= capv, axis=1)
        cand = np.flatnonzero(fits)
        if cand.size:
            rem = (slots[cand] - used[cand]).astype(np.float64)
            perslot = (capv - S[cand] - di).min(1) / np.maximum(rem - 1, 0.5)
            w = cand[np.argmax(perslot)]
        else:
            bfree = np.flatnonzero(used[nsmall:] < slots[nsmall:]) + nsmall
            if bfree.size == 0:
                cand2 = np.flatnonzero(used < slots)
                add = (-(-(S[cand2] + di) // wsize) - (-(-S[cand2] // wsize))).sum(1)
                w = cand2[np.argmin(add)]
            else:
                w = bfree[np.argmin(S[bfree].max(1))]
        w_of[i] = w
        slot_of[i] = used[w]
        S[w] += di
        used[w] += 1
    return w_of, slot_of


def _preprocess(X, weight, degrees, row_pointers, column_index):
    row_ids = _row_ids_from_pointers(row_pointers)          # [E] sorted, in [0,N)
    col = column_index.astype(np.int64)
    deg = np.ascontiguousarray(degrees.astype(np.float32))

    core = row_ids // NPC                                   # [E] in [0,8)
    local = row_ids - core * NPC
    q = col // CHUNK                                        # [E] in [0,4)

    # per-(core, local row, chunk) edge counts for window steering
    dkey = (core * NPC + local) * NQ + q
    dmat = np.bincount(dkey, minlength=NCORES * NPC * NQ).reshape(NCORES, NPC, NQ)
    W_OF = np.empty((NCORES, NPC), np.int64)
    SLOT_OF = np.empty((NCORES, NPC), np.int64)
    for c in range(NCORES):
        W_OF[c], SLOT_OF[c] = _steer_windows(dmat[c])

    w_local = W_OF[core, local]                             # [E] in [0,98)
    dstrel_all = SLOT_OF[core, local].astype(np.float32)
    src16_all = (col - q * CHUNK).astype(np.int16)

    key = ((core * NQ + q) * NW + w_local).astype(np.int64)  # (c, q, w)
    counts = np.bincount(key, minlength=NCORES * NQ * NW).reshape(NCORES, NQ, NW)
    t_qw = -(-counts.max(axis=0) // P)                       # [NQ, NW]
    # no chunk may have an empty stream (zero-size params break AP lowering);
    # a pad tile (src=0, dstrel=-1) contributes nothing
    for qq in range(NQ):
        if t_qw[qq].sum() == 0:
            t_qw[qq, 0] = 1
    lq = t_qw.sum(axis=1) * P                                # [NQ] stream lengths
    chunk_base = np.concatenate([np.zeros(1, np.int64), np.cumsum(lq)])
    ltot = int(chunk_base[-1])
    # offset of window w's padded segment within chunk q's stream
    offs_qw = np.cumsum(np.concatenate([np.zeros((NQ, 1), np.int64), t_qw[:, :-1]], axis=1) * P, axis=1) \
        if False else (np.cumsum(t_qw, axis=1) - t_qw) * P   # [NQ, NW] exclusive prefix

    order = np.argsort(key, kind="stable")
    key_s = key[order]
    starts_flat = np.concatenate([np.zeros(1, np.int64), np.cumsum(counts.reshape(-1))])[:-1]
    rank_s = np.arange(E, dtype=np.int64) - starts_flat[key_s]
    q_s = (key_s // NW) % NQ
    w_s = key_s % NW
    core_s = key_s // (NQ * NW)
    pos_s = chunk_base[q_s] + offs_qw[q_s, w_s] + rank_s     # [E] position in core's array

    src_pad = np.zeros((NCORES, ltot), np.int16)
    dstrel_pad = np.full((NCORES, ltot), -1.0, np.float32)
    src_pad[core_s, pos_s] = src16_all[order]
    dstrel_pad[core_s, pos_s] = dstrel_all[order]

    # per-chunk device layouts
    idx_w, dst_t = [], []
    for qq in range(NQ):
        sl = slice(int(chunk_base[qq]), int(chunk_base[qq + 1]))
        s = src_pad[:, sl]                                   # [NC, LQ]
        # wrapped idx layout [128, LQ/16]: idx i at [i%16, i//16], replicated 8x
        iw = np.tile(s.reshape(NCORES, -1, 16).transpose(0, 2, 1), (1, 8, 1))
        idx_w.append(np.ascontiguousarray(iw))
        dst_t.append(np.ascontiguousarray(
            dstrel_pad[:, sl].reshape(NCORES, -1, P).transpose(0, 2, 1).astype(np.float16)))

    # per-core dest-degree table [P, NW] in steered window order
    degt = np.zeros((NCORES, P, NW), np.float32)
    for c in range(NCORES):
        degt[c, SLOT_OF[c], W_OF[c]] = deg[c * NPC : (c + 1) * NPC]
    # map original local row -> position in the core's steered output
    pos = W_OF * P + SLOT_OF                                # [NCORES, NPC]

    # fold the source-degree normalization into the gather table (standard
    # GCN norm precompute, same class as the reference's rsqrt degrees):
    # gathered rows arrive pre-scaled, so sel is a pure one-hot (1 DVE op).
    xt = np.ascontiguousarray((deg[:, None] * X).astype(np.float16))
    w16 = np.ascontiguousarray(weight.astype(np.float16))
    t_key = tuple(tuple(int(x) for x in row) for row in t_qw)
    return xt, w16, idx_w, dst_t, degt, pos, t_key


SB_T = int(os.environ.get("GCN_SB_T", "16"))  # tiles per batched sel build


def _build_program(t_qw):
    import concourse.bacc as bacc
    import concourse.bass as bass
    import concourse.mybir as mybir
    import concourse.tile as tile

    lq = [sum(t_qw[q]) * P for q in range(NQ)]

    nc = bacc.Bacc(
        "TRN2", target_bir_lowering=False, num_swdge_queues=4,
        # descriptor-ring carveout: 2x the default so a queue can hold two
        # gather calls' descriptors -> descgen of call n+1 overlaps drain of n
        dynamic_dma_scratch_size=int(os.environ.get("GCN_DDS", "65536")),
    )
    xt_p = nc.declare_dram_parameter("xt", [N, F], mybir.dt.float16, isOutput=False)
    idx_ps = [nc.declare_dram_parameter(f"idx{q}", [P, lq[q] // 16], mybir.dt.int16, isOutput=False) for q in range(NQ)]
    dst_ps = [nc.declare_dram_parameter(f"dstrel{q}", [P, lq[q] // P], mybir.dt.float16, isOutput=False) for q in range(NQ)]
    degt_p = nc.declare_dram_parameter("degt", [P, NW], mybir.dt.float32, isOutput=False)
    w_p = nc.declare_dram_parameter("w16", [F, F], mybir.dt.float16, isOutput=False)
    out_p = nc.declare_dram_parameter("out", [NW * P, F], mybir.dt.float32, isOutput=True)

    def bcast_mid(ap, t):
        # [128, t] AP -> [128, t, F] with stride-0 inner (value per (p, tile))
        return bass.AP(ap.tensor, ap.offset, [ap.ap[0], [ap.ap[1][0], t], [0, F]])

    with tile.TileContext(nc) as tc:
        with (
            tc.tile_pool(name="persist", bufs=1) as persist,
            tc.tile_pool(name="gblk", bufs=int(os.environ.get("GCN_GBUFS", "3"))) as gpool,
            tc.tile_pool(name="selp", bufs=int(os.environ.get("GCN_SBUFS", "2"))) as selpool,
            tc.tile_pool(name="atsb", bufs=2) as atpool,
            tc.tile_pool(name="outsb", bufs=2) as outpool,
            tc.tile_pool(name="psum1", bufs=2, space="PSUM") as psum1,
            tc.tile_pool(name="psum2", bufs=2, space="PSUM") as psum2,
        ):
            # Stage idx/dst/ds in call-aligned pieces so the first gather of
            # each queue only waits on a small initial load, not the full
            # ~1MB table (ramp was ~29us with whole-table loads).
            IDX_PIECE = GB_TILES * P // 16 * 8      # 8 gather-calls per piece
            SEL_PIECE = SB_T * 16                   # 16 sel-batches per piece
            idx_sb, dst_sb = [], []
            for q in range(NQ):
                idx_sb.append(persist.tile([P, lq[q] // 16], mybir.dt.int16,
                                           tag=f"idx{q}", name=f"idx{q}"))
                dst_sb.append(persist.tile([P, lq[q] // P], mybir.dt.float16,
                                           tag=f"dst{q}", name=f"dst{q}"))
            np_idx = max(-(-(lq[q] // 16) // IDX_PIECE) for q in range(NQ))
            np_sel = max(-(-(lq[q] // P) // SEL_PIECE) for q in range(NQ))
            for pi in range(max(np_idx, np_sel)):
                for q in range(NQ):
                    a, b = pi * IDX_PIECE, min((pi + 1) * IDX_PIECE, lq[q] // 16)
                    if a < b:
                        nc.sync.dma_start(idx_sb[q][:, a:b], idx_ps[q][:, a:b])
                    a, b = pi * SEL_PIECE, min((pi + 1) * SEL_PIECE, lq[q] // P)
                    if a < b:
                        nc.sync.dma_start(dst_sb[q][:, a:b], dst_ps[q][:, a:b])
            degt_sb = persist.tile([P, NW], mybir.dt.float32)
            nc.sync.dma_start(degt_sb[:], degt_p[:])
            w_sb = persist.tile([F, F], mybir.dt.float16)
            nc.sync.dma_start(w_sb[:], w_p[:])
            c_i32 = persist.tile([P, P], mybir.dt.int32)
            nc.gpsimd.iota(c_i32[:], pattern=[[1, P]], base=0, channel_multiplier=0)
            c_f16 = persist.tile([P, P], mybir.dt.float16)
            nc.vector.tensor_copy(c_f16[:], c_i32[:])
            zero_sb = persist.tile([P, F], mybir.dt.float32)
            nc.vector.memset(zero_sb[:], 0.0)

            pos = [0] * NQ
            gblk = [None] * NQ
            selblk = [None] * NQ
            for w in range(NW):
                ntiles_w = sum(t_qw[q][w] for q in range(NQ))
                if ntiles_w == 0:
                    nc.sync.dma_start(out=out_p[w * P : (w + 1) * P, :], in_=zero_sb[:])
                    continue
                at_ps = psum1.tile([F, P], mybir.dt.float32, space="PSUM")
                k = 0
                for q in range(NQ):
                    for _t in range(t_qw[q][w]):
                        if pos[q] % GB_TILES == 0:
                            nt_call = min(GB_TILES, lq[q] // P - pos[q])
                            nidx = nt_call * P
                            gblk[q] = gpool.tile(
                                [P, GB_TILES * F], mybir.dt.float16,
                                tag=f"gblk{q}", name=f"gblk{q}",
                            )
                            nc.gpsimd.dma_gather(
                                out_ap=gblk[q][:, : nt_call * F].rearrange(
                                    "p (k f) -> p k f", f=F
                                ),
                                in_ap=xt_p[q * CHUNK : (q + 1) * CHUNK, :],
                                idxs_ap=idx_sb[q][:, pos[q] * P // 16 : (pos[q] * P + nidx) // 16],
                                num_idxs=nidx,
                                num_idxs_reg=nidx,
                                elem_size=F,
                                queue_num=q,
                                single_packet=(os.environ.get('GCN_SP','0')=='1'),
                            )
                        if pos[q] % SB_T == 0:
                            nt_s = min(SB_T, lq[q] // P - pos[q])
                            selblk[q] = selpool.tile(
                                [P, SB_T * F], mybir.dt.float16,
                                tag=f"sel{q}", name=f"sel{q}",
                            )
                            c_b = bass.AP(c_f16[:].tensor, c_f16[:].offset,
                                          [c_f16[:].ap[0], [0, nt_s], [1, F]])
                            nc.vector.tensor_tensor(
                                out=selblk[q][:, : nt_s * F].rearrange("p (t f) -> p t f", f=F),
                                in0=c_b,
                                in1=bcast_mid(dst_sb[q][:, pos[q] : pos[q] + nt_s], nt_s),
                                op=mybir.AluOpType.is_equal,
                            )
                        j = pos[q] % GB_TILES
                        js = pos[q] % SB_T
                        nc.tensor.matmul(
                            out=at_ps[:],
                            lhsT=gblk[q][:, j * F : (j + 1) * F],
                            rhs=selblk[q][:, js * F : (js + 1) * F],
                            start=(k == 0),
                            stop=(k == ntiles_w - 1),
                        )
                        pos[q] += 1
                        k += 1
                at_sb = atpool.tile([F, P], mybir.dt.float16)
                nc.scalar.activation(at_sb[:], at_ps[:], mybir.ActivationFunctionType.Copy)
                o2_ps = psum2.tile([P, F], mybir.dt.float32, space="PSUM")
                nc.tensor.matmul(out=o2_ps[:], lhsT=at_sb[:], rhs=w_sb[:], start=True, stop=True)
                outsb = outpool.tile([P, F], mybir.dt.float32)
                nc.scalar.activation(outsb[:], o2_ps[:], mybir.ActivationFunctionType.Copy,
                                     scale=degt_sb[:, w : w + 1])
                nc.sync.dma_start(out=out_p[w * P : (w + 1) * P, :], in_=outsb[:])
    nc.compile()
    return nc


def _get_program(t_key):
    key = (t_key, GB_TILES, SB_T, os.environ.get("GCN_SP", "0"),
           os.environ.get("GCN_DDS", ""), os.environ.get("GCN_GBUFS", ""))
    if key not in _PROGRAM_CACHE:
        _PROGRAM_CACHE[key] = _build_program(t_key)
    return _PROGRAM_CACHE[key]


def _run(nc, in_maps, trace=False, **kw):
    from concourse.bass_utils import run_bass_kernel_spmd

    return run_bass_kernel_spmd(nc, in_maps, core_ids=list(range(NCORES)),
                                trace=trace, **kw)


def kernel(X, weight, degrees, row_pointers, column_index, _trace=False, _ret_raw=False):
    assert X.shape == (N, F) and column_index.shape == (E,)
    xt, w16, idx_w, dst_t, degt, pos, t_key = _preprocess(
        X, weight, degrees, row_pointers, column_index
    )
    nc = _get_program(t_key)
    in_maps = []
    for c in range(NCORES):
        m = {"xt": xt, "degt": degt[c], "w16": w16}
        for q in range(NQ):
            m[f"idx{q}"] = idx_w[q][c]
            m[f"dstrel{q}"] = dst_t[q][c]
        in_maps.append(m)
    res = _run(nc, in_maps, trace=_trace)
    out = np.empty((N, F), np.float32)
    for c in range(NCORES):
        out[c * NPC : (c + 1) * NPC] = res.results[c]["out"][pos[c]]
    if _ret_raw:
        return out, res
    return out



# revision 21
# speedup vs baseline: 1.0670x; 1.0119x over previous
"""GCN forward (gather + segment-sum + matmul) on 8 TRN2 NeuronCores.

Algorithm (factorized GCN):
    out[i] = deg[i] * (sum_{j in N(i)} deg[j] * X[j]) @ W

Sharding: destination nodes are split across the 8 cores (12500 rows each);
the fp16 feature table Y = deg[:,None]*X (GCN norm folded in, like the
reference's precomputed rsqrt degrees) is replicated to every core's HBM.
Each core:
  - gathers the fp16 rows of Y for its ~200K edges with gpsimd dma_gather
    (the memory-bound part; int16 indices force a 4-way chunking of the
    100K-row table, so each core keeps 4 chunk-local edge streams),
  - window membership is STEERED on the host (_steer_windows) so per-(chunk,
    window) edge counts pack tightly into 4x128 tiles across all 8 SPMD
    cores (~11% padding vs ~25% for contiguous windows),
  - builds one-hot selection matrices sel[e,d] = (dstrel[e] == d) in batches
    of 16 tiles with one broadcast-AP DVE is_equal,
  - segment-sums via TensorE: A_T[f,d] += G[e,f]^T @ sel[e,d], accumulating
    in PSUM over all of a 128-destination window's edge tiles (across the 4
    chunk streams),
  - applies W with a second matmul and scales rows by deg_dest,
  - writes its 12544-row slice; the host unpermutes the steered windows.

The aggregation, both matmuls and the dest-degree scaling happen on device;
the host computes indices/partitioning/normalization staging.
"""
import os

import numpy as np

N = 100000
E = 1600000
F = 128
P = 128
NCORES = 8
NPC = N // NCORES          # 12500 destination rows per core
NW = (NPC + P - 1) // P    # 98 windows of 128 destinations per core
NQ = 4                     # table chunks (int16 gather indices)
CHUNK = 25000              # rows per chunk
GB_TILES = int(os.environ.get("GCN_GB_TILES", "16"))  # tiles per gather call
# dma_gather per-call num_idxs is capped by the SWDGE descriptor-ring carveout,
# sized via Bacc(dynamic_dma_scratch_size=...): 16*128=2048 idxs (128 descs/lane)
# is safe with the 64KB carveout below; bigger calls amortize the ~1us/call Q7
# launch and the ~2-3us inter-drain bubble on each of the 4 SWDGE queues.

_PROGRAM_CACHE: dict = {}


def _row_ids_from_pointers(row_pointers: np.ndarray) -> np.ndarray:
    """Replicates jnp.repeat(arange(N), diff(rp), total_repeat_length=E)."""
    rl = np.diff(row_pointers.astype(np.int64))
    starts = np.concatenate([np.zeros(1, np.int64), np.cumsum(rl)[:-1]])
    return np.searchsorted(starts, np.arange(E, dtype=np.int64), side="right") - 1


NBIG = int(os.environ.get("GCN_NBIG", "24"))  # overflow windows for steering


def _steer_windows(dmat, nbig=NBIG, nw=NW, wsize=P):
    """Assign destination rows to windows so per-(chunk, window) edge counts
    pack tightly under 4*128, minimizing SPMD tile padding. Small windows are
    hard-capped at 4 tiles; overflow rows go to the trailing big windows."""
    npc = dmat.shape[0]
    slots = np.full(nw, wsize, np.int64)
    slots[-1] -= nw * wsize - npc
    nsmall = nw - nbig
    capv = 4 * wsize
    order = np.argsort(-dmat.sum(1), kind="stable")
    S = np.zeros((nw, NQ), np.int64)
    used = np.zeros(nw, np.int64)
    w_of = np.empty(npc, np.int64)
    slot_of = np.empty(npc, np.int64)
    for i in order:
        di = dmat[i]
        frees = used[:nsmall] < slots[:nsmall]
        fits = frees & np.all(S[:nsmall] + di <= capv, axis=1)
        cand = np.flatnonzero(fits)
        if cand.size:
            rem = (slots[cand] - used[cand]).astype(np.float64)
            perslot = (capv - S[cand] - di).min(1) / np.maximum(rem - 1, 0.5)
            w = cand[np.argmax(perslot)]
        else:
            bfree = np.flatnonzero(used[nsmall:] < slots[nsmall:]) + nsmall
            if bfree.size == 0:
                cand2 = np.flatnonzero(used < slots)
                add = (-(-(S[cand2] + di) // wsize) - (-(-S[cand2] // wsize))).sum(1)
                w = cand2[np.argmin(add)]
            else:
                w = bfree[np.argmin(S[bfree].max(1))]
        w_of[i] = w
        slot_of[i] = used[w]
        S[w] += di
        used[w] += 1
    return w_of, slot_of


def _preprocess(X, weight, degrees, row_pointers, column_index):
    row_ids = _row_ids_from_pointers(row_pointers)          # [E] sorted, in [0,N)
    col = column_index.astype(np.int64)
    deg = np.ascontiguousarray(degrees.astype(np.float32))

    core = row_ids // NPC                                   # [E] in [0,8)
    local = row_ids - core * NPC
    q = col // CHUNK                                        # [E] in [0,4)

    # per-(core, local row, chunk) edge counts for window steering
    dkey = (core * NPC + local) * NQ + q
    dmat = np.bincount(dkey, minlength=NCORES * NPC * NQ).reshape(NCORES, NPC, NQ)
    W_OF = np.empty((NCORES, NPC), np.int64)
    SLOT_OF = np.empty((NCORES, NPC), np.int64)
    for c in range(NCORES):
        W_OF[c], SLOT_OF[c] = _steer_windows(dmat[c])

    w_local = W_OF[core, local]                             # [E] in [0,98)
    dstrel_all = SLOT_OF[core, local].astype(np.float32)
    src16_all = (col - q * CHUNK).astype(np.int16)

    key = ((core * NQ + q) * NW + w_local).astype(np.int64)  # (c, q, w)
    counts = np.bincount(key, minlength=NCORES * NQ * NW).reshape(NCORES, NQ, NW)
    t_qw = -(-counts.max(axis=0) // P)                       # [NQ, NW]
    # no chunk may have an empty stream (zero-size params break AP lowering);
    # a pad tile (src=0, dstrel=-1) contributes nothing
    for qq in range(NQ):
        if t_qw[qq].sum() == 0:
            t_qw[qq, 0] = 1
    lq = t_qw.sum(axis=1) * P                                # [NQ] stream lengths
    chunk_base = np.concatenate([np.zeros(1, np.int64), np.cumsum(lq)])
    ltot = int(chunk_base[-1])
    # offset of window w's padded segment within chunk q's stream
    offs_qw = np.cumsum(np.concatenate([np.zeros((NQ, 1), np.int64), t_qw[:, :-1]], axis=1) * P, axis=1) \
        if False else (np.cumsum(t_qw, axis=1) - t_qw) * P   # [NQ, NW] exclusive prefix

    order = np.argsort(key, kind="stable")
    key_s = key[order]
    starts_flat = np.concatenate([np.zeros(1, np.int64), np.cumsum(counts.reshape(-1))])[:-1]
    rank_s = np.arange(E, dtype=np.int64) - starts_flat[key_s]
    q_s = (key_s // NW) % NQ
    w_s = key_s % NW
    core_s = key_s // (NQ * NW)
    pos_s = chunk_base[q_s] + offs_qw[q_s, w_s] + rank_s     # [E] position in core's array

    src_pad = np.zeros((NCORES, ltot), np.int16)
    dstrel_pad = np.full((NCORES, ltot), -1.0, np.float32)
    src_pad[core_s, pos_s] = src16_all[order]
    dstrel_pad[core_s, pos_s] = dstrel_all[order]

    # per-chunk device layouts
    idx_w, dst_t = [], []
    for qq in range(NQ):
        sl = slice(int(chunk_base[qq]), int(chunk_base[qq + 1]))
        s = src_pad[:, sl]                                   # [NC, LQ]
        # wrapped idx layout [128, LQ/16]: idx i at [i%16, i//16], replicated 8x
        iw = np.tile(s.reshape(NCORES, -1, 16).transpose(0, 2, 1), (1, 8, 1))
        idx_w.append(np.ascontiguousarray(iw))
        dst_t.append(np.ascontiguousarray(
            dstrel_pad[:, sl].reshape(NCORES, -1, P).transpose(0, 2, 1).astype(np.float16)))

    # per-core dest-degree table [P, NW] in steered window order
    degt = np.zeros((NCORES, P, NW), np.float32)
    for c in range(NCORES):
        degt[c, SLOT_OF[c], W_OF[c]] = deg[c * NPC : (c + 1) * NPC]
    # map original local row -> position in the core's steered output
    pos = W_OF * P + SLOT_OF                                # [NCORES, NPC]

    # fold the source-degree normalization into the gather table (standard
    # GCN norm precompute, same class as the reference's rsqrt degrees):
    # gathered rows arrive pre-scaled, so sel is a pure one-hot (1 DVE op).
    xt = np.ascontiguousarray((deg[:, None] * X).astype(np.float16))
    w16 = np.ascontiguousarray(weight.astype(np.float16))
    t_key = tuple(tuple(int(x) for x in row) for row in t_qw)
    return xt, w16, idx_w, dst_t, degt, pos, t_key


SB_T = int(os.environ.get("GCN_SB_T", "16"))  # tiles per batched sel build


def _build_program(t_qw):
    import concourse.bacc as bacc
    import concourse.bass as bass
    import concourse.mybir as mybir
    import concourse.tile as tile

    lq = [sum(t_qw[q]) * P for q in range(NQ)]

    nc = bacc.Bacc(
        "TRN2", target_bir_lowering=False, num_swdge_queues=4,
        # descriptor-ring carveout: 2x the default so a queue can hold two
        # gather calls' descriptors -> descgen of call n+1 overlaps drain of n
        dynamic_dma_scratch_size=int(os.environ.get("GCN_DDS", "65536")),
    )
    xt_p = nc.declare_dram_parameter("xt", [N, F], mybir.dt.float16, isOutput=False)
    idx_ps = [nc.declare_dram_parameter(f"idx{q}", [P, lq[q] // 16], mybir.dt.int16, isOutput=False) for q in range(NQ)]
    dst_ps = [nc.declare_dram_parameter(f"dstrel{q}", [P, lq[q] // P], mybir.dt.float16, isOutput=False) for q in range(NQ)]
    degt_p = nc.declare_dram_parameter("degt", [P, NW], mybir.dt.float32, isOutput=False)
    w_p = nc.declare_dram_parameter("w16", [F, F], mybir.dt.float16, isOutput=False)
    out_p = nc.declare_dram_parameter("out", [NW * P, F], mybir.dt.float32, isOutput=True)

    def bcast_mid(ap, t):
        # [128, t] AP -> [128, t, F] with stride-0 inner (value per (p, tile))
        return bass.AP(ap.tensor, ap.offset, [ap.ap[0], [ap.ap[1][0], t], [0, F]])

    with tile.TileContext(nc) as tc:
        with (
            tc.tile_pool(name="persist", bufs=1) as persist,
            tc.tile_pool(name="gblk", bufs=int(os.environ.get("GCN_GBUFS", "3"))) as gpool,
            tc.tile_pool(name="selp", bufs=int(os.environ.get("GCN_SBUFS", "2"))) as selpool,
            tc.tile_pool(name="atsb", bufs=2) as atpool,
            tc.tile_pool(name="outsb", bufs=2) as outpool,
            tc.tile_pool(name="psum1", bufs=2, space="PSUM") as psum1,
            tc.tile_pool(name="psum2", bufs=2, space="PSUM") as psum2,
        ):
            # Stage idx/dst/ds in call-aligned pieces so the first gather of
            # each queue only waits on a small initial load, not the full
            # ~1MB table (ramp was ~29us with whole-table loads).
            IDX_PIECE = GB_TILES * P // 16 * 8      # 8 gather-calls per piece
            SEL_PIECE = SB_T * 16                   # 16 sel-batches per piece
            idx_sb, dst_sb = [], []
            for q in range(NQ):
                idx_sb.append(persist.tile([P, lq[q] // 16], mybir.dt.int16,
                                           tag=f"idx{q}", name=f"idx{q}"))
                dst_sb.append(persist.tile([P, lq[q] // P], mybir.dt.float16,
                                           tag=f"dst{q}", name=f"dst{q}"))
            np_idx = max(-(-(lq[q] // 16) // IDX_PIECE) for q in range(NQ))
            np_sel = max(-(-(lq[q] // P) // SEL_PIECE) for q in range(NQ))
            # piece 0: all four queues' idx pieces FIRST so every queue's
            # first gather can launch ASAP, then the dst pieces for sel
            for q in range(NQ):
                b = min(IDX_PIECE, lq[q] // 16)
                nc.sync.dma_start(idx_sb[q][:, :b], idx_ps[q][:, :b])
            for q in range(NQ):
                b = min(SEL_PIECE, lq[q] // P)
                nc.sync.dma_start(dst_sb[q][:, :b], dst_ps[q][:, :b])
            for pi in range(1, max(np_idx, np_sel)):
                for q in range(NQ):
                    a, b = pi * IDX_PIECE, min((pi + 1) * IDX_PIECE, lq[q] // 16)
                    if a < b:
                        nc.sync.dma_start(idx_sb[q][:, a:b], idx_ps[q][:, a:b])
                    a, b = pi * SEL_PIECE, min((pi + 1) * SEL_PIECE, lq[q] // P)
                    if a < b:
                        nc.sync.dma_start(dst_sb[q][:, a:b], dst_ps[q][:, a:b])
            degt_sb = persist.tile([P, NW], mybir.dt.float32)
            nc.sync.dma_start(degt_sb[:], degt_p[:])
            w_sb = persist.tile([F, F], mybir.dt.float16)
            nc.sync.dma_start(w_sb[:], w_p[:])
            c_i32 = persist.tile([P, P], mybir.dt.int32)
            nc.gpsimd.iota(c_i32[:], pattern=[[1, P]], base=0, channel_multiplier=0)
            c_f16 = persist.tile([P, P], mybir.dt.float16)
            nc.vector.tensor_copy(c_f16[:], c_i32[:])
            zero_sb = persist.tile([P, F], mybir.dt.float32)
            nc.vector.memset(zero_sb[:], 0.0)

            pos = [0] * NQ
            gblk = [None] * NQ
            selblk = [None] * NQ
            for w in range(NW):
                ntiles_w = sum(t_qw[q][w] for q in range(NQ))
                if ntiles_w == 0:
                    nc.sync.dma_start(out=out_p[w * P : (w + 1) * P, :], in_=zero_sb[:])
                    continue
                at_ps = psum1.tile([F, P], mybir.dt.float32, space="PSUM")
                k = 0
                for q in range(NQ):
                    for _t in range(t_qw[q][w]):
                        if pos[q] % GB_TILES == 0:
                            nt_call = min(GB_TILES, lq[q] // P - pos[q])
                            nidx = nt_call * P
                            gblk[q] = gpool.tile(
                                [P, GB_TILES * F], mybir.dt.float16,
                                tag=f"gblk{q}", name=f"gblk{q}",
                            )
                            nc.gpsimd.dma_gather(
                                out_ap=gblk[q][:, : nt_call * F].rearrange(
                                    "p (k f) -> p k f", f=F
                                ),
                                in_ap=xt_p[q * CHUNK : (q + 1) * CHUNK, :],
                                idxs_ap=idx_sb[q][:, pos[q] * P // 16 : (pos[q] * P + nidx) // 16],
                                num_idxs=nidx,
                                num_idxs_reg=nidx,
                                elem_size=F,
                                queue_num=q,
                                single_packet=(os.environ.get('GCN_SP','0')=='1'),
                            )
                        if pos[q] % SB_T == 0:
                            nt_s = min(SB_T, lq[q] // P - pos[q])
                            selblk[q] = selpool.tile(
                                [P, SB_T * F], mybir.dt.float16,
                                tag=f"sel{q}", name=f"sel{q}",
                            )
                            c_b = bass.AP(c_f16[:].tensor, c_f16[:].offset,
                                          [c_f16[:].ap[0], [0, nt_s], [1, F]])
                            nc.vector.tensor_tensor(
                                out=selblk[q][:, : nt_s * F].rearrange("p (t f) -> p t f", f=F),
                                in0=c_b,
                                in1=bcast_mid(dst_sb[q][:, pos[q] : pos[q] + nt_s], nt_s),
                                op=mybir.AluOpType.is_equal,
                            )
                        j = pos[q] % GB_TILES
                        js = pos[q] % SB_T
                        nc.tensor.matmul(
                            out=at_ps[:],
                            lhsT=gblk[q][:, j * F : (j + 1) * F],
                            rhs=selblk[q][:, js * F : (js + 1) * F],
                            start=(k == 0),
                            stop=(k == ntiles_w - 1),
                        )
                        pos[q] += 1
                        k += 1
                at_sb = atpool.tile([F, P], mybir.dt.float16)
                nc.scalar.activation(at_sb[:], at_ps[:], mybir.ActivationFunctionType.Copy)
                o2_ps = psum2.tile([P, F], mybir.dt.float32, space="PSUM")
                nc.tensor.matmul(out=o2_ps[:], lhsT=at_sb[:], rhs=w_sb[:], start=True, stop=True)
                outsb = outpool.tile([P, F], mybir.dt.float32)
                nc.scalar.activation(outsb[:], o2_ps[:], mybir.ActivationFunctionType.Copy,
                                     scale=degt_sb[:, w : w + 1])
                nc.sync.dma_start(out=out_p[w * P : (w + 1) * P, :], in_=outsb[:])
    nc.compile()
    return nc


def _get_program(t_key):
    key = (t_key, GB_TILES, SB_T, os.environ.get("GCN_SP", "0"),
           os.environ.get("GCN_DDS", ""), os.environ.get("GCN_GBUFS", ""))
    if key not in _PROGRAM_CACHE:
        _PROGRAM_CACHE[key] = _build_program(t_key)
    return _PROGRAM_CACHE[key]


def _run(nc, in_maps, trace=False, **kw):
    from concourse.bass_utils import run_bass_kernel_spmd

    return run_bass_kernel_spmd(nc, in_maps, core_ids=list(range(NCORES)),
                                trace=trace, **kw)


def kernel(X, weight, degrees, row_pointers, column_index, _trace=False, _ret_raw=False):
    assert X.shape == (N, F) and column_index.shape == (E,)
    xt, w16, idx_w, dst_t, degt, pos, t_key = _preprocess(
        X, weight, degrees, row_pointers, column_index
    )
    nc = _get_program(t_key)
    in_maps = []
    for c in range(NCORES):
        m = {"xt": xt, "degt": degt[c], "w16": w16}
        for q in range(NQ):
            m[f"idx{q}"] = idx_w[q][c]
            m[f"dstrel{q}"] = dst_t[q][c]
        in_maps.append(m)
    res = _run(nc, in_maps, trace=_trace)
    out = np.empty((N, F), np.float32)
    for c in range(NCORES):
        out[c * NPC : (c + 1) * NPC] = res.results[c]["out"][pos[c]]
    if _ret_raw:
        return out, res
    return out

